# revision 31
# baseline (speedup 1.0000x reference)
"""Per-frame RMS energy (STFT framing: n_fft=1024, hop=256, center/reflect pad)
over a [16, 1048576] f32 signal -> [16, 4096, 1] f32.

Trainium2 Bass/Tile kernel, data-parallel over batch across 8 NeuronCores
(2 signals per core). Each 1024-sample frame is exactly 4 consecutive
256-sample hop blocks, so we compute per-block sums of squares (one read of
every input byte -> memory-bound optimal), then a sliding sum of 4 plus
sqrt(mean).

Layout: partition p of a signal owns frames p*32..p*32+31; its input row is
the naturally aligned x[p*8192 : (p+1)*8192]. ext[p, u] = s_pad[p*32+u]
(u in 0..34) where s_pad[b] is the padded-signal 256-block sum of squares;
cols 2..33 come from grouped reduces, the 3-value seam from the neighbor
partition via two tiny SBUF->SBUF DMAs, and the reflect-pad edge sums via
fused square+accumulate on ACT (the tail edge is loaded straight into
partition 127 so its accum lands in ext[127,34] with no extra copy).

Perf shape (v2): ALL bulk loads are issued up front with a dedicated SBUF
buffer per chunk (per-size tile tags, bufs=count) so the load stream is
never issue-gated by compute -- the v1 trace showed the HWDGE queue running
dry mid-stream waiting on ACT to free tin slots. The first two loads ride
the Scalar HWDGE ring (its sequencer exits the preamble ~1.4us before
Sync's); the rest ride Sync. Output stores ride Sync AFTER all loads so
they cannot head-of-line-block the load stream. The last signal's stream
ends in two 128KiB chunks consumed by one fused DVE square+reduce each,
keeping the post-stream dependency chain short. The ~7us epilogue (the
backend compiler's 253-semaphore restore chains) is fixed overhead.
"""

import sys
import types

import numpy as np

import concourse.bacc as bacc
import concourse.bass as bass
import concourse.mybir as mybir
import concourse.tile as tile
from concourse.bass_utils import run_bass_kernel_spmd
from concourse.vector_clock import ScopedClock


def _install_ntff_hook_shim():
    """The image's antenv lacks axon_hooks; if a caller turns on tracing
    (e.g. via BASS_TRACE=1), run_bass_kernel_spmd imports it. Provide the
    ctypes-based hook so that path works instead of raising."""
    try:
        import antenv.axon_hooks  # noqa: F401

        return
    except ImportError:
        pass
    try:
        from trn_agent_boot.trn_boot import _ntff_profile_via_ctypes

        hook = _ntff_profile_via_ctypes("/opt/axon/libaxon_pjrt.so")
    except Exception:
        hook = None
    mod = types.ModuleType("antenv.axon_hooks")
    mod.get_axon_ntff_profile_hook = lambda: hook
    mod.set_axon_ntff_profile_hook = lambda h: None
    sys.modules["antenv.axon_hooks"] = mod


_install_ntff_hook_shim()




class SlimExitTileContext(tile.TileContext):
    """TileContext whose exit sequence drops the second all-engine barrier.

    The stock epilogue is drain -> barrier -> sem clear -> barrier. The
    first barrier guarantees every engine is idle before the gpsimd range
    sem-clear runs; the trailing barrier only re-synchronizes engines that
    are each about to run off the end of their own queues, so skipping it
    is safe (NRT completion still waits for every queue, and the sem state
    a re-execution needs is restored by the clear).
    """

    def _drain_and_barrier(self, tick_clock, wait_clock):
        # Single Pool-side rendezvous: gpsimd waits out the full vector clock
        # (all compute retired, all DMA receipts landed) and then resets sem
        # state. No all-engine barrier at all: every other engine's queue
        # simply ends after its last real instruction, so the per-engine
        # event-semaphore restore chains the toolchain appends run early,
        # overlapped with the stream, instead of serialized after a barrier.
        drain_inst = self.nc.gpsimd.drain()
        wait_clock.add_sem_waits(
            drain_inst.ins, ScopedClock({None: tick_clock.global_clock})
        )
        assert self.sems is not None
        popped = self.nc._tile_sem_poison_stack.pop()
        assert popped is self._sem_poison
        self.nc.clear_and_free_semaphores(list(self.sems.allocated().values()))


# Problem constants (self-contained; must match the grader's input spec)
B = 16                 # signals in the batch
T = 1048576            # samples per signal
N_FFT = 1024
HOP = 256
N_CORES = 8
SIG_PER_CORE = B // N_CORES   # 2
P = 128                       # SBUF partitions
NBLK = T // HOP               # 4096 hop blocks per signal
CPB = NBLK // P               # 32 output frames per partition
SPP = T // P                  # 8192 samples per partition row
NFRAMES = NBLK                # 4096 output frames per signal

# Per-signal chunks of the 8192-sample partition row, in 256-blocks
# (block_offset, n_blocks, ring). The (28,4) chunk goes first in both
# signals: it writes ext cols 32:34, the seam-copy source, so the
# cross-partition DMAs resolve mid-stream. Signal 1 (the last to stream)
# tapers into small chunks so the post-stream dependency chain is short.
#
# Loads alternate between the Scalar ('A') and Sync ('B') HWDGE rings,
# byte-balanced so both rings drain together: one ring alone sustains only
# ~330 GB/s while two keep the 16 SDMA engines at the ~358 GB/s HBM
# pair-share (engines round-robin between rings at packet granularity,
# covering each ring's per-packet receipt stalls). The SWDGE (gpsimd) ring
# is NOT used for bulk: v3 measured it ramping ~5us late and moving only
# ~150 GB/s. Both rings taper into the small tail chunks.
CHUNKS0 = [(28, 4, "A"), (0, 8, "B"), (8, 8, "A"), (16, 8, "B"), (24, 4, "A")]
CHUNKS1 = [(28, 4, "B"), (0, 8, "B"), (8, 4, "B"), (12, 4, "B"), (16, 4, "B"),
           (20, 4, "B"), (24, 2, "B"), (26, 1, "B"), (27, 1, "B")]
ALL_CHUNKS = [CHUNKS0, CHUNKS1]

F32 = mybir.dt.float32
BF16 = mybir.dt.bfloat16
AF = mybir.ActivationFunctionType
AX = mybir.AxisListType
ADD = mybir.AluOpType.add
MUL = mybir.AluOpType.mult


def build_bass():
    # Bacc (not plain Bass): its compile pipeline splits multi-sem waits into
    # event-semaphore instructions, which this walrus build requires.
    #
    # Bass.__init__ ends with an all-engine barrier whose only job is to
    # order its const-AP memsets against const-AP readers. This kernel reads
    # no const APs (every activation gets an explicit zeros-tile bias that
    # Tile orders itself), so skip that barrier: it otherwise delays the
    # first load DMA behind the slowest engine's instruction fetch.
    orig_barrier = bass.Bass.all_engine_barrier
    bass.Bass.all_engine_barrier = lambda self, *, sem_only=False: None
    try:
        nc = bacc.Bacc()
    finally:
        bass.Bass.all_engine_barrier = orig_barrier
    x = nc.dram_tensor("signal", [SIG_PER_CORE, T], F32, kind="ExternalInput")
    y = nc.dram_tensor("out", [SIG_PER_CORE, NFRAMES], F32, kind="ExternalOutput")

    xr = x[:, :].rearrange("b (p f) -> b p f", p=P)   # [2, 128, 8192]
    yr = y[:, :].rearrange("b (p c) -> b p c", p=P)   # [2, 128, 32]

    with SlimExitTileContext(nc) as tc:
        with (
            tc.tile_pool(name="in8", bufs=4) as in8_pool,
            tc.tile_pool(name="in4", bufs=8) as in4_pool,
            tc.tile_pool(name="in21", bufs=1) as in21_pool,
            tc.tile_pool(name="sq8", bufs=4) as sq8_pool,
            tc.tile_pool(name="sq4", bufs=8) as sq4_pool,
            tc.tile_pool(name="sq21", bufs=1) as sq21_pool,
            tc.tile_pool(name="ext", bufs=2) as ext_pool,
            tc.tile_pool(name="spec", bufs=2) as spec_pool,
            tc.tile_pool(name="small", bufs=2) as small_pool,
        ):
            def in_tile(nb, tag):
                if nb == 8:
                    return in8_pool.tile([P, 8 * HOP], F32, tag="t8", name="t8")
                if nb == 4:
                    return in4_pool.tile([P, 4 * HOP], F32, tag="t4", name="t4")
                return in21_pool.tile([P, nb * HOP], F32, tag=tag, name=tag)

            # (bf16 square outputs were tried: the DVE grouped reduce ran at
            # the same 2286ns/2048elem as f32 -- no 16-bit speedup on
            # TENSOR_REDUCE -- so keep f32 for the tighter numerics.)
            def sq_tile(nb, tag):
                if nb == 8:
                    return sq8_pool.tile([P, 8 * HOP], F32, tag="q8", name="q8")
                if nb == 4:
                    return sq4_pool.tile([P, 4 * HOP], F32, tag="q4", name="q4")
                return sq21_pool.tile([P, nb * HOP], F32, tag=tag, name=tag)

            # Zeros tile used as the explicit activation bias everywhere, so
            # no instruction reads the constructor-time const APs (see the
            # barrier note in build_bass). Tile orders memset vs readers.
            zb = spec_pool.tile([P, 1], F32, tag="zb")
            nc.vector.memset(zb[:, :], 0.0)

            # Phase 0: tiny edge loads for both signals (SWDGE, overlap the
            # stream). spc row 0 = [x[1:257], x[257:513], x[T-257:T-1]].
            spcs = []
            for sig in range(SIG_PER_CORE):
                spc = spec_pool.tile([P, 768], F32, tag="spc")
                # Pinned at ~10us sim so the edge squares (their consumers)
                # schedule behind the first bulk squares on ACT instead of
                # head-of-line blocking on these slow tiny SWDGE loads.
                with tc.tile_wait_until(0.010):
                    nc.gpsimd.dma_start(
                        out=spc[0:1, 0:512], in_=x[sig : sig + 1, 1:513]
                    )
                    nc.gpsimd.dma_start(
                        out=spc[0:1, 512:768], in_=x[sig : sig + 1, T - 257 : T - 1]
                    )
                spcs.append(spc)

            # Phase A: issue EVERY bulk load up front. Each chunk owns a
            # dedicated SBUF buffer (per-size tags, bufs=count) so no load
            # waits on compute. Global issue order is pinned A1,B1,A2,B2,...
            # via tile_wait_until: the first 8 HWDGE DMAs then take the 8
            # fresh completion-sem lanes, and each later load recycles the
            # lane of an early-completing predecessor (the Tile scheduler
            # otherwise reorders same-queue DMAs, which in one iteration put
            # 1 MiB loads behind lane waits that resolved at 14-23us and
            # starved the ring).
            # The Tile scheduler orders each engine queue by CoreSim-
            # simulated readiness, and CoreSim's DMA model knows nothing of
            # the real ~179 GB/s/ring HBM share -- left alone it decides the
            # tail chunks "arrive" early and schedules their consumers ahead
            # of the bulk reduces, which then head-of-line block on the real
            # late arrivals (measured +8..13us). Pin every load at its
            # predicted real arrival so the sim's world matches reality and
            # every consumer is scheduled in true arrival order. The pins
            # are scheduler-side only: runtime pacing stays semaphores.
            tins = [[None] * len(ALL_CHUNKS[sig]) for sig in range(SIG_PER_CORE)]
            ring_lists = {"A": [], "B": []}
            for sig in range(SIG_PER_CORE):
                for ci, (b0, nb, ring) in enumerate(ALL_CHUNKS[sig]):
                    ring_lists[ring].append((sig, ci, b0, nb))
            # A-ring (2 head chunks) unpinned: they sit at the front of
            # ACT's queue before any compute. B-ring loads are pinned at a
            # blended-arrival model (single busy ring ~330 GB/s behind A's
            # 1.5 MB head start).
            # Pin at the consumption-order global arrival estimate: both
            # rings drain ~equally while both hold content (~210 GB/s
            # each), then the survivor takes the full ~330-420.
            pins = {}
            cum = 0.0
            for sig in range(SIG_PER_CORE):
                for ci, (b0, nb, ring) in enumerate(ALL_CHUNKS[sig]):
                    cum += nb * HOP * P * 4 / 1.0e6
                    pins[(sig, ci)] = max(0.0, (cum / 360.0e-6) * 1e-6 - 0.002)
            order = [("A", c) for c in ring_lists["A"]]
            order += [("B", c) for c in ring_lists["B"]]
            for ring, (sig, ci, b0, nb) in order:
                ln = nb * HOP
                tin = in_tile(nb, tag=f"t{nb}_{ci}")
                eng = nc.scalar if ring == "A" else nc.sync
                with tc.tile_wait_until(pins[(sig, ci)]):
                    eng.dma_start(
                        out=tin[:, :],
                        in_=xr[sig, :, b0 * HOP : b0 * HOP + ln],
                    )
                tins[sig][ci] = tin

            # Dummy Sqrt so the ACT table set covering Square and Sqrt loads
            # once, up front, instead of a ~1.3us reload mid-stream.
            dummy = spec_pool.tile([1, 1], F32, tag="dummy")
            nc.vector.memset(dummy[0:1, 0:1], 1.0)
            nc.scalar.activation(
                out=dummy[0:1, 0:1], in_=dummy[0:1, 0:1], func=AF.Sqrt,
                bias=zb[0:1, 0:1],
            )

            # Per-signal pipeline: squares/reduces/seams, then edge sums,
            # then windows+sqrt+store. Signal 0's whole pipeline (including
            # its store) completes mid-stream; only signal 1's short tail
            # trails the last load.
            for sig in range(SIG_PER_CORE):
                ext = ext_pool.tile([P, 36], F32, name="ext")

                # Square (ACT) + 256-block reduce (DVE) for nb>=2 chunks;
                # fused DVE square+reduce for the 1-block tails. Seam copies
                # are emitted right after the chunk that writes their source
                # columns (Tile only tracks dependencies on already-emitted
                # writers).
                for ci, (b0, nb, ring) in enumerate(ALL_CHUNKS[sig]):
                    ln = nb * HOP
                    tin = tins[sig][ci]
                    tsq = sq_tile(nb, tag=f"q{nb}_{ci}")
                    if nb == 1:
                        # Fused square+block-sum in ONE DVE op (the Pool
                        # engine rejects TensorScalarPtr on TRN2, so these
                        # cannot be offloaded there).
                        nc.vector.scalar_tensor_tensor(
                            out=tsq[:, :], in0=tin[:, :], scalar=1.0,
                            in1=tin[:, :], op0=MUL, op1=MUL,
                            accum_out=ext[:, 2 + b0 : 2 + b0 + 1],
                        )
                    else:
                        nc.scalar.activation(
                            out=tsq[:, :], in_=tin[:, :], func=AF.Square,
                            bias=zb[:, 0:1],
                        )
                        nc.vector.tensor_reduce(
                            out=ext[:, 2 + b0 : 2 + b0 + nb],
                            in_=tsq[:, :].rearrange("p (g k) -> p g k", k=HOP),
                            axis=AX.X,
                            op=ADD,
                        )
                    if b0 == 28:
                        # ext[p, 0:2] = s_pad[p*32 .. +1] = ext[p-1, 32:34]
                        nc.gpsimd.dma_start(
                            out=ext[1:128, 0:2], in_=ext[0:127, 32:34]
                        )
                    elif b0 == 0:
                        # ext[p, 34] = s_pad[p*32+34] = ext[p+1, 2]
                        nc.gpsimd.dma_start(
                            out=ext[0:127, 34:35], in_=ext[1:128, 2:3]
                        )

                # Reflect-pad edge sums, emitted AFTER this signal's bulk
                # squares: the spc SWDGE loads complete ~15-17us, and edge
                # ops emitted early head-of-line block ACT's FIFO behind
                # those completions (measured: ACT idle until ~17us, +8us
                # end-to-end). Here they slot in when ACT is already paced
                # by chunk arrivals, and the spr copy still clears gpsimd
                # well before this signal's window adds:
                #   s_pad[1]    = sum x[1:257]^2     -> ext[0, 1]
                #   s_pad[0]    = sum x[257:513]^2   -> ext[0, 0]
                #   s_pad[4098] = sum x[T-257:T-1]^2 -> ext[127, 34] (spr)
                spc = spcs[sig]
                spq = spec_pool.tile([P, 768], F32, tag="spq")
                spr = spec_pool.tile([P, 1], F32, tag="spr")
                nc.scalar.activation(
                    out=spq[0:1, 0:256], in_=spc[0:1, 0:256], func=AF.Square,
                    bias=zb[0:1, 0:1], accum_out=ext[0:1, 1:2],
                )
                nc.scalar.activation(
                    out=spq[0:1, 256:512], in_=spc[0:1, 256:512], func=AF.Square,
                    bias=zb[0:1, 0:1], accum_out=ext[0:1, 0:1],
                )
                nc.scalar.activation(
                    out=spq[0:1, 512:768], in_=spc[0:1, 512:768], func=AF.Square,
                    bias=zb[0:1, 0:1], accum_out=spr[0:1, 0:1],
                )
                nc.gpsimd.dma_start(out=ext[127:128, 34:35], in_=spr[0:1, 0:1])

                # Window-of-4 sums + sqrt(mean) + output.
                # E[p, c] = ext[p, c] + ... + ext[p, c+3], via pairwise
                # sums: P1[c] = ext[c] + ext[c+1]; E[c] = P1[c] + P1[c+2].
                # Stores ride Sync, emitted after every load, so they
                # cannot head-of-line block the load stream.
                p1 = small_pool.tile([P, 34], F32, tag="p1")
                e1 = small_pool.tile([P, CPB], F32, tag="e1")
                nc.vector.tensor_add(out=p1[:, :], in0=ext[:, 0:34], in1=ext[:, 1:35])
                nc.vector.tensor_add(out=e1[:, :], in0=p1[:, 0:32], in1=p1[:, 2:34])
                ot = small_pool.tile([P, CPB], F32, tag="ot")
                nc.scalar.activation(
                    out=ot[:, :], in_=e1[:, :], func=AF.Sqrt, scale=1.0 / N_FFT,
                    bias=zb[:, 0:1],
                )
                # Store via Sync HWDGE: its queue is pure loads, and the
                # arrival pins make the scheduler place stores after every
                # load (sim-ready later than all load pins), so no
                # head-of-line risk; HWDGE receipt beats SWDGE by ~0.7us.
                nc.sync.dma_start(out=yr[sig, :, :], in_=ot[:, :])
    nc.finalize()
    return nc


_NC = None


def run(signal: np.ndarray, trace: bool = False):
    global _NC
    sig = np.ascontiguousarray(np.asarray(signal, dtype=np.float32))
    assert sig.shape == (B, T), sig.shape
    if _NC is None:
        _NC = build_bass()
    in_maps = [
        {"signal": np.ascontiguousarray(sig[k * SIG_PER_CORE : (k + 1) * SIG_PER_CORE])}
        for k in range(N_CORES)
    ]
    res = run_bass_kernel_spmd(_NC, in_maps, core_ids=list(range(N_CORES)), trace=trace)
    out = np.concatenate([r["out"] for r in res.results], axis=0)
    return out.reshape(B, NFRAMES, 1).astype(np.float32), res


def kernel(signal: np.ndarray) -> np.ndarray:
    out, _ = run(signal, trace=False)
    return out


# revision 32
# speedup vs baseline: 1.0047x; 1.0047x over previous
"""Per-frame RMS energy (STFT framing: n_fft=1024, hop=256, center/reflect pad)
over a [16, 1048576] f32 signal -> [16, 4096, 1] f32.

Trainium2 Bass/Tile kernel, data-parallel over batch across 8 NeuronCores
(2 signals per core). Each 1024-sample frame is exactly 4 consecutive
256-sample hop blocks, so we compute per-block sums of squares (one read of
every input byte -> memory-bound optimal), then a sliding sum of 4 plus
sqrt(mean).

Layout: partition p of a signal owns frames p*32..p*32+31; its input row is
the naturally aligned x[p*8192 : (p+1)*8192]. ext[p, u] = s_pad[p*32+u]
(u in 0..34) where s_pad[b] is the padded-signal 256-block sum of squares;
cols 2..33 come from grouped reduces, the 3-value seam from the neighbor
partition via two tiny SBUF->SBUF DMAs, and the reflect-pad edge sums via
fused square+accumulate on ACT (the tail edge is loaded straight into
partition 127 so its accum lands in ext[127,34] with no extra copy).

Perf shape (v2): ALL bulk loads are issued up front with a dedicated SBUF
buffer per chunk (per-size tile tags, bufs=count) so the load stream is
never issue-gated by compute -- the v1 trace showed the HWDGE queue running
dry mid-stream waiting on ACT to free tin slots. The first two loads ride
the Scalar HWDGE ring (its sequencer exits the preamble ~1.4us before
Sync's); the rest ride Sync. Output stores ride Sync AFTER all loads so
they cannot head-of-line-block the load stream. The last signal's stream
ends in two 128KiB chunks consumed by one fused DVE square+reduce each,
keeping the post-stream dependency chain short. The ~7us epilogue (the
backend compiler's 253-semaphore restore chains) is fixed overhead.
"""

import sys
import types

import numpy as np

import concourse.bacc as bacc
import concourse.bass as bass
import concourse.mybir as mybir
import concourse.tile as tile
from concourse.bass_utils import run_bass_kernel_spmd
from concourse.vector_clock import ScopedClock


def _install_ntff_hook_shim():
    """The image's antenv lacks axon_hooks; if a caller turns on tracing
    (e.g. via BASS_TRACE=1), run_bass_kernel_spmd imports it. Provide the
    ctypes-based hook so that path works instead of raising."""
    try:
        import antenv.axon_hooks  # noqa: F401

        return
    except ImportError:
        pass
    try:
        from trn_agent_boot.trn_boot import _ntff_profile_via_ctypes

        hook = _ntff_profile_via_ctypes("/opt/axon/libaxon_pjrt.so")
    except Exception:
        hook = None
    mod = types.ModuleType("antenv.axon_hooks")
    mod.get_axon_ntff_profile_hook = lambda: hook
    mod.set_axon_ntff_profile_hook = lambda h: None
    sys.modules["antenv.axon_hooks"] = mod


_install_ntff_hook_shim()




class SlimExitTileContext(tile.TileContext):
    """TileContext whose exit sequence drops the second all-engine barrier.

    The stock epilogue is drain -> barrier -> sem clear -> barrier. The
    first barrier guarantees every engine is idle before the gpsimd range
    sem-clear runs; the trailing barrier only re-synchronizes engines that
    are each about to run off the end of their own queues, so skipping it
    is safe (NRT completion still waits for every queue, and the sem state
    a re-execution needs is restored by the clear).
    """

    def _drain_and_barrier(self, tick_clock, wait_clock):
        # Single Pool-side rendezvous: gpsimd waits out the full vector clock
        # (all compute retired, all DMA receipts landed) and then resets sem
        # state. No all-engine barrier at all: every other engine's queue
        # simply ends after its last real instruction, so the per-engine
        # event-semaphore restore chains the toolchain appends run early,
        # overlapped with the stream, instead of serialized after a barrier.
        drain_inst = self.nc.gpsimd.drain()
        wait_clock.add_sem_waits(
            drain_inst.ins, ScopedClock({None: tick_clock.global_clock})
        )
        assert self.sems is not None
        popped = self.nc._tile_sem_poison_stack.pop()
        assert popped is self._sem_poison
        self.nc.clear_and_free_semaphores(list(self.sems.allocated().values()))


# Problem constants (self-contained; must match the grader's input spec)
B = 16                 # signals in the batch
T = 1048576            # samples per signal
N_FFT = 1024
HOP = 256
N_CORES = 8
SIG_PER_CORE = B // N_CORES   # 2
P = 128                       # SBUF partitions
NBLK = T // HOP               # 4096 hop blocks per signal
CPB = NBLK // P               # 32 output frames per partition
SPP = T // P                  # 8192 samples per partition row
NFRAMES = NBLK                # 4096 output frames per signal

# Per-signal chunks of the 8192-sample partition row, in 256-blocks
# (block_offset, n_blocks, ring). The (28,4) chunk goes first in both
# signals: it writes ext cols 32:34, the seam-copy source, so the
# cross-partition DMAs resolve mid-stream. Signal 1 (the last to stream)
# tapers into small chunks so the post-stream dependency chain is short.
#
# Loads alternate between the Scalar ('A') and Sync ('B') HWDGE rings,
# byte-balanced so both rings drain together: one ring alone sustains only
# ~330 GB/s while two keep the 16 SDMA engines at the ~358 GB/s HBM
# pair-share (engines round-robin between rings at packet granularity,
# covering each ring's per-packet receipt stalls). The SWDGE (gpsimd) ring
# is NOT used for bulk: v3 measured it ramping ~5us late and moving only
# ~150 GB/s. Both rings taper into the small tail chunks.
CHUNKS0 = [(28, 4, "A"), (0, 8, "A"), (8, 8, "B"), (16, 8, "B"), (24, 4, "B")]
CHUNKS1 = [(28, 4, "B"), (0, 8, "B"), (8, 4, "B"), (12, 4, "B"), (16, 4, "B"),
           (20, 4, "B"), (24, 2, "B"), (26, 1, "B"), (27, 1, "B")]
ALL_CHUNKS = [CHUNKS0, CHUNKS1]

F32 = mybir.dt.float32
BF16 = mybir.dt.bfloat16
AF = mybir.ActivationFunctionType
AX = mybir.AxisListType
ADD = mybir.AluOpType.add
MUL = mybir.AluOpType.mult


def build_bass():
    # Bacc (not plain Bass): its compile pipeline splits multi-sem waits into
    # event-semaphore instructions, which this walrus build requires.
    #
    # Bass.__init__ ends with an all-engine barrier whose only job is to
    # order its const-AP memsets against const-AP readers. This kernel reads
    # no const APs (every activation gets an explicit zeros-tile bias that
    # Tile orders itself), so skip that barrier: it otherwise delays the
    # first load DMA behind the slowest engine's instruction fetch.
    orig_barrier = bass.Bass.all_engine_barrier
    bass.Bass.all_engine_barrier = lambda self, *, sem_only=False: None
    try:
        nc = bacc.Bacc()
    finally:
        bass.Bass.all_engine_barrier = orig_barrier
    x = nc.dram_tensor("signal", [SIG_PER_CORE, T], F32, kind="ExternalInput")
    y = nc.dram_tensor("out", [SIG_PER_CORE, NFRAMES], F32, kind="ExternalOutput")

    xr = x[:, :].rearrange("b (p f) -> b p f", p=P)   # [2, 128, 8192]
    yr = y[:, :].rearrange("b (p c) -> b p c", p=P)   # [2, 128, 32]

    with SlimExitTileContext(nc) as tc:
        with (
            tc.tile_pool(name="in8", bufs=4) as in8_pool,
            tc.tile_pool(name="in4", bufs=8) as in4_pool,
            tc.tile_pool(name="in21", bufs=1) as in21_pool,
            tc.tile_pool(name="sq8", bufs=4) as sq8_pool,
            tc.tile_pool(name="sq4", bufs=8) as sq4_pool,
            tc.tile_pool(name="sq21", bufs=1) as sq21_pool,
            tc.tile_pool(name="ext", bufs=2) as ext_pool,
            tc.tile_pool(name="spec", bufs=2) as spec_pool,
            tc.tile_pool(name="small", bufs=2) as small_pool,
        ):
            def in_tile(nb, tag):
                if nb == 8:
                    return in8_pool.tile([P, 8 * HOP], F32, tag="t8", name="t8")
                if nb == 4:
                    return in4_pool.tile([P, 4 * HOP], F32, tag="t4", name="t4")
                return in21_pool.tile([P, nb * HOP], F32, tag=tag, name=tag)

            # (bf16 square outputs were tried: the DVE grouped reduce ran at
            # the same 2286ns/2048elem as f32 -- no 16-bit speedup on
            # TENSOR_REDUCE -- so keep f32 for the tighter numerics.)
            def sq_tile(nb, tag):
                if nb == 8:
                    return sq8_pool.tile([P, 8 * HOP], F32, tag="q8", name="q8")
                if nb == 4:
                    return sq4_pool.tile([P, 4 * HOP], F32, tag="q4", name="q4")
                return sq21_pool.tile([P, nb * HOP], F32, tag=tag, name=tag)

            # Zeros tile used as the explicit activation bias everywhere, so
            # no instruction reads the constructor-time const APs (see the
            # barrier note in build_bass). Tile orders memset vs readers.
            zb = spec_pool.tile([P, 1], F32, tag="zb")
            nc.vector.memset(zb[:, :], 0.0)

            # Phase 0: tiny edge loads for both signals (SWDGE, overlap the
            # stream). spc row 0 = [x[1:257], x[257:513], x[T-257:T-1]].
            spcs = []
            for sig in range(SIG_PER_CORE):
                spc = spec_pool.tile([P, 768], F32, tag="spc")
                # Pinned at ~10us sim so the edge squares (their consumers)
                # schedule behind the first bulk squares on ACT instead of
                # head-of-line blocking on these slow tiny SWDGE loads.
                with tc.tile_wait_until(0.010):
                    nc.gpsimd.dma_start(
                        out=spc[0:1, 0:512], in_=x[sig : sig + 1, 1:513]
                    )
                    nc.gpsimd.dma_start(
                        out=spc[0:1, 512:768], in_=x[sig : sig + 1, T - 257 : T - 1]
                    )
                spcs.append(spc)

            # Phase A: issue EVERY bulk load up front. Each chunk owns a
            # dedicated SBUF buffer (per-size tags, bufs=count) so no load
            # waits on compute. Global issue order is pinned A1,B1,A2,B2,...
            # via tile_wait_until: the first 8 HWDGE DMAs then take the 8
            # fresh completion-sem lanes, and each later load recycles the
            # lane of an early-completing predecessor (the Tile scheduler
            # otherwise reorders same-queue DMAs, which in one iteration put
            # 1 MiB loads behind lane waits that resolved at 14-23us and
            # starved the ring).
            # The Tile scheduler orders each engine queue by CoreSim-
            # simulated readiness, and CoreSim's DMA model knows nothing of
            # the real ~179 GB/s/ring HBM share -- left alone it decides the
            # tail chunks "arrive" early and schedules their consumers ahead
            # of the bulk reduces, which then head-of-line block on the real
            # late arrivals (measured +8..13us). Pin every load at its
            # predicted real arrival so the sim's world matches reality and
            # every consumer is scheduled in true arrival order. The pins
            # are scheduler-side only: runtime pacing stays semaphores.
            tins = [[None] * len(ALL_CHUNKS[sig]) for sig in range(SIG_PER_CORE)]
            ring_lists = {"A": [], "B": []}
            for sig in range(SIG_PER_CORE):
                for ci, (b0, nb, ring) in enumerate(ALL_CHUNKS[sig]):
                    ring_lists[ring].append((sig, ci, b0, nb))
            # A-ring (2 head chunks) unpinned: they sit at the front of
            # ACT's queue before any compute. B-ring loads are pinned at a
            # blended-arrival model (single busy ring ~330 GB/s behind A's
            # 1.5 MB head start).
            # A-ring (2 head chunks) unpinned: they sit at the front of
            # ACT's queue before any compute. B-ring loads are pinned at a
            # blended-arrival model (single busy ring ~330 GB/s behind A's
            # 1.5 MB head start).
            B_GBPS = 330.0e-6      # MB per ns
            pins = {}
            cum = 1.5
            for sig, ci, b0, nb in ring_lists["B"]:
                mb = nb * HOP * P * 4 / 1.0e6
                cum += mb
                pins[(sig, ci)] = max(0.0, (cum / B_GBPS) * 1e-6 - 0.002)
            for sig, ci, b0, nb in ring_lists["A"]:
                pins[(sig, ci)] = 0.0
            order = [("A", c) for c in ring_lists["A"]]
            order += [("B", c) for c in ring_lists["B"]]
            for ring, (sig, ci, b0, nb) in order:
                ln = nb * HOP
                tin = in_tile(nb, tag=f"t{nb}_{ci}")
                eng = nc.scalar if ring == "A" else nc.sync
                with tc.tile_wait_until(pins[(sig, ci)]):
                    eng.dma_start(
                        out=tin[:, :],
                        in_=xr[sig, :, b0 * HOP : b0 * HOP + ln],
                    )
                tins[sig][ci] = tin

            # Dummy Sqrt so the ACT table set covering Square and Sqrt loads
            # once, up front, instead of a ~1.3us reload mid-stream.
            dummy = spec_pool.tile([1, 1], F32, tag="dummy")
            nc.vector.memset(dummy[0:1, 0:1], 1.0)
            nc.scalar.activation(
                out=dummy[0:1, 0:1], in_=dummy[0:1, 0:1], func=AF.Sqrt,
                bias=zb[0:1, 0:1],
            )

            # Per-signal pipeline: squares/reduces/seams, then edge sums,
            # then windows+sqrt+store. Signal 0's whole pipeline (including
            # its store) completes mid-stream; only signal 1's short tail
            # trails the last load.
            for sig in range(SIG_PER_CORE):
                ext = ext_pool.tile([P, 36], F32, name="ext")

                # Square (ACT) + 256-block reduce (DVE) for nb>=2 chunks;
                # fused DVE square+reduce for the 1-block tails. Seam copies
                # are emitted right after the chunk that writes their source
                # columns (Tile only tracks dependencies on already-emitted
                # writers).
                for ci, (b0, nb, ring) in enumerate(ALL_CHUNKS[sig]):
                    ln = nb * HOP
                    tin = tins[sig][ci]
                    tsq = sq_tile(nb, tag=f"q{nb}_{ci}")
                    if nb == 1:
                        # Fused square+block-sum in ONE DVE op (the Pool
                        # engine rejects TensorScalarPtr on TRN2, so these
                        # cannot be offloaded there).
                        nc.vector.scalar_tensor_tensor(
                            out=tsq[:, :], in0=tin[:, :], scalar=1.0,
                            in1=tin[:, :], op0=MUL, op1=MUL,
                            accum_out=ext[:, 2 + b0 : 2 + b0 + 1],
                        )
                    else:
                        nc.scalar.activation(
                            out=tsq[:, :], in_=tin[:, :], func=AF.Square,
                            bias=zb[:, 0:1],
                        )
                        nc.vector.tensor_reduce(
                            out=ext[:, 2 + b0 : 2 + b0 + nb],
                            in_=tsq[:, :].rearrange("p (g k) -> p g k", k=HOP),
                            axis=AX.X,
                            op=ADD,
                        )
                    if b0 == 28:
                        # ext[p, 0:2] = s_pad[p*32 .. +1] = ext[p-1, 32:34]
                        nc.gpsimd.dma_start(
                            out=ext[1:128, 0:2], in_=ext[0:127, 32:34]
                        )
                    elif b0 == 0:
                        # ext[p, 34] = s_pad[p*32+34] = ext[p+1, 2]
                        nc.gpsimd.dma_start(
                            out=ext[0:127, 34:35], in_=ext[1:128, 2:3]
                        )

                # Reflect-pad edge sums, emitted AFTER this signal's bulk
                # squares: the spc SWDGE loads complete ~15-17us, and edge
                # ops emitted early head-of-line block ACT's FIFO behind
                # those completions (measured: ACT idle until ~17us, +8us
                # end-to-end). Here they slot in when ACT is already paced
                # by chunk arrivals, and the spr copy still clears gpsimd
                # well before this signal's window adds:
                #   s_pad[1]    = sum x[1:257]^2     -> ext[0, 1]
                #   s_pad[0]    = sum x[257:513]^2   -> ext[0, 0]
                #   s_pad[4098] = sum x[T-257:T-1]^2 -> ext[127, 34] (spr)
                spc = spcs[sig]
                spq = spec_pool.tile([P, 768], F32, tag="spq")
                spr = spec_pool.tile([P, 1], F32, tag="spr")
                nc.scalar.activation(
                    out=spq[0:1, 0:256], in_=spc[0:1, 0:256], func=AF.Square,
                    bias=zb[0:1, 0:1], accum_out=ext[0:1, 1:2],
                )
                nc.scalar.activation(
                    out=spq[0:1, 256:512], in_=spc[0:1, 256:512], func=AF.Square,
                    bias=zb[0:1, 0:1], accum_out=ext[0:1, 0:1],
                )
                nc.scalar.activation(
                    out=spq[0:1, 512:768], in_=spc[0:1, 512:768], func=AF.Square,
                    bias=zb[0:1, 0:1], accum_out=spr[0:1, 0:1],
                )
                nc.gpsimd.dma_start(out=ext[127:128, 34:35], in_=spr[0:1, 0:1])

                # Window-of-4 sums + sqrt(mean) + output.
                # E[p, c] = ext[p, c] + ... + ext[p, c+3], via pairwise
                # sums: P1[c] = ext[c] + ext[c+1]; E[c] = P1[c] + P1[c+2].
                # Stores ride Sync, emitted after every load, so they
                # cannot head-of-line block the load stream.
                p1 = small_pool.tile([P, 34], F32, tag="p1")
                e1 = small_pool.tile([P, CPB], F32, tag="e1")
                nc.vector.tensor_add(out=p1[:, :], in0=ext[:, 0:34], in1=ext[:, 1:35])
                nc.vector.tensor_add(out=e1[:, :], in0=p1[:, 0:32], in1=p1[:, 2:34])
                ot = small_pool.tile([P, CPB], F32, tag="ot")
                nc.scalar.activation(
                    out=ot[:, :], in_=e1[:, :], func=AF.Sqrt, scale=1.0 / N_FFT,
                    bias=zb[:, 0:1],
                )
                # Store via Sync HWDGE: its queue is pure loads, and the
                # arrival pins make the scheduler place stores after every
                # load (sim-ready later than all load pins), so no
                # head-of-line risk; HWDGE receipt beats SWDGE by ~0.7us.
                nc.sync.dma_start(out=yr[sig, :, :], in_=ot[:, :])
    nc.finalize()
    return nc


_NC = None


def run(signal: np.ndarray, trace: bool = False):
    global _NC
    sig = np.ascontiguousarray(np.asarray(signal, dtype=np.float32))
    assert sig.shape == (B, T), sig.shape
    if _NC is None:
        _NC = build_bass()
    in_maps = [
        {"signal": np.ascontiguousarray(sig[k * SIG_PER_CORE : (k + 1) * SIG_PER_CORE])}
        for k in range(N_CORES)
    ]
    res = run_bass_kernel_spmd(_NC, in_maps, core_ids=list(range(N_CORES)), trace=trace)
    out = np.concatenate([r["out"] for r in res.results], axis=0)
    return out.reshape(B, NFRAMES, 1).astype(np.float32), res


def kernel(signal: np.ndarray) -> np.ndarray:
    out, _ = run(signal, trace=False)
    return out
